# revision 1
# baseline (speedup 1.0000x reference)
"""Bass/TRN2 kernel for nn_EnvCollLoss (oriented-footprint raster collision loss).

Strategy: agents are sharded by map index across 8 cores (2 cores per map).
Each core holds its map as a Y8-bitpacked fp16 ap_gather table in SBUF
(partition j holds raster columns x===j mod 16; index e=(ix//16)*128+(iy//16)
returns the two 8-row packed words covering rows [16*(iy//16), +16)).
Per point: gather word-pair + a one-hot pair (by ix%16), mask-multiply,
block-diagonal matmul reduces the 16 candidate partitions, transpose-DMA back
to state-major, integer bit-extract, masked argmin via Max/MaxIndex, penalty.
"""
import sys
import types
import numpy as np
from contextlib import ExitStack

NA, T = 256, 100
N_MAPS, MAP_H, MAP_W = 4, 2048, 2048
PU, PV = 10, 20
P = PU * PV  # 200
N_CORES = 8

# jnp.linspace(-0.5, 0.5, 10/20, dtype=float32) exact values (validated vs jax)
_UU10 = np.array([-0.5, -0.3888889, -0.2777778, -0.16666667, -0.05555556,
                  0.05555556, 0.16666667, 0.2777778, 0.3888889, 0.5], dtype=np.float32)
_VV20 = np.linspace(-0.5, 0.5, 20, dtype=np.float32)


def _install_ntff_hook():
    import antenv
    if "antenv.axon_hooks" in sys.modules:
        return
    try:
        from trn_agent_boot.trn_boot import _ntff_profile_via_ctypes
        hook = _ntff_profile_via_ctypes("/opt/axon/libaxon_pjrt.so")
    except Exception:
        hook = None
    mod = types.ModuleType("antenv.axon_hooks")
    mod._hook = hook
    mod.get_axon_ntff_profile_hook = lambda: mod._hook
    mod.set_axon_ntff_profile_hook = lambda h: setattr(mod, "_hook", h)
    sys.modules["antenv.axon_hooks"] = mod
    antenv.axon_hooks = mod


_PROGRAM_CACHE = {}


def _build_program(n_tiles):
    import concourse.tile as tile
    from concourse import bacc, mybir

    dt = mybir.dt
    A = mybir.AluOpType

    nc = bacc.Bacc("TRN2", target_bir_lowering=False, debug=False,
                   enable_asserts=False, num_devices=N_CORES)
    S = n_tiles * 128

    tab_in = nc.dram_tensor("tab", [128, 16384 * 2], dt.float16, kind="ExternalInput").ap()
    mt_in = nc.dram_tensor("mt", [128, 32 * 2], dt.float16, kind="ExternalInput").ap()
    ones_in = nc.dram_tensor("ones8", [128, 8], dt.float16, kind="ExternalInput").ap()
    uu_in = nc.dram_tensor("uu", [128, P], dt.float32, kind="ExternalInput").ap()
    vv_in = nc.dram_tensor("vv", [128, P], dt.float32, kind="ExternalInput").ap()
    io_in = nc.dram_tensor("iotaf", [128, P], dt.float32, kind="ExternalInput").ap()
    dx_in = nc.dram_tensor("dxrep", [128, 1], dt.float32, kind="ExternalInput").ap()
    traj_in = nc.dram_tensor("trajsh", [S, 4], dt.float32, kind="ExternalInput").ap()
    att_in = nc.dram_tensor("attsh", [S, 2], dt.float32, kind="ExternalInput").ap()
    out_dram = nc.dram_tensor("outsh", [n_tiles, 128], dt.float32, kind="ExternalOutput").ap()

    with tile.TileContext(nc) as tc, ExitStack() as ctx:
        cpool = ctx.enter_context(tc.tile_pool(name="const", bufs=1))
        wpool = ctx.enter_context(tc.tile_pool(name="work", bufs=2))
        gpool = ctx.enter_context(tc.tile_pool(name="gath", bufs=2))
        spool = ctx.enter_context(tc.tile_pool(name="stgp", bufs=1))
        ppool = ctx.enter_context(tc.tile_pool(name="ps", bufs=8, space="PSUM"))

        tab = cpool.tile([128, 16384 * 2], dt.float16)
        nc.sync.dma_start(tab[:], tab_in)
        mt = cpool.tile([128, 64], dt.float16)
        nc.sync.dma_start(mt[:], mt_in)
        ones8 = cpool.tile([128, 8], dt.float16)
        nc.sync.dma_start(ones8[:], ones_in)
        uu = cpool.tile([128, P], dt.float32)
        nc.sync.dma_start(uu[:], uu_in)
        vv = cpool.tile([128, P], dt.float32)
        nc.sync.dma_start(vv[:], vv_in)
        iotaf = cpool.tile([128, P], dt.float32)
        nc.sync.dma_start(iotaf[:], io_in)
        dxrep = cpool.tile([128, 1], dt.float32)
        nc.sync.dma_start(dxrep[:], dx_in)
        invdx = cpool.tile([128, 1], dt.float32)
        nc.vector.reciprocal(invdx[:], dxrep[:])

        for it in range(n_tiles):
            tr = wpool.tile([128, 4], dt.float32, tag="tr")
            nc.sync.dma_start(tr[:], traj_in[it * 128:(it + 1) * 128, :])
            at = wpool.tile([128, 2], dt.float32, tag="at")
            nc.sync.dma_start(at[:], att_in[it * 128:(it + 1) * 128, :])
            cx, cy = tr[:, 0:1], tr[:, 1:2]
            hx0, hy0 = tr[:, 2:3], tr[:, 3:4]
            Lat, Wat = at[:, 0:1], at[:, 1:2]

            # den = sqrt(hx0^2+hy0^2) via near-1 closed form; hn = h * recip(den)
            ps = wpool.tile([128, 8], dt.float32, tag="ps")
            nc.vector.tensor_tensor(ps[:, 0:1], hx0, hx0, A.mult)
            nc.vector.tensor_tensor(ps[:, 1:2], hy0, hy0, A.mult)
            nc.vector.tensor_tensor(ps[:, 2:3], ps[:, 0:1], ps[:, 1:2], A.add)  # x
            nc.vector.tensor_scalar(ps[:, 3:4], ps[:, 2:3], -1.0, None, A.add)  # delta
            nc.vector.tensor_scalar(ps[:, 4:5], ps[:, 3:4], 0.5, None, A.mult)
            nc.vector.tensor_tensor(ps[:, 5:6], ps[:, 3:4], ps[:, 3:4], A.mult)
            nc.vector.tensor_scalar(ps[:, 5:6], ps[:, 5:6], -0.125, None, A.mult)
            nc.vector.tensor_tensor(ps[:, 4:5], ps[:, 4:5], ps[:, 5:6], A.add)
            nc.vector.tensor_scalar(ps[:, 4:5], ps[:, 4:5], 1.0, None, A.add)   # den
            inv = wpool.tile([128, 1], dt.float32, tag="inv")
            nc.vector.reciprocal(inv[:], ps[:, 4:5])
            hh = wpool.tile([128, 4], dt.float32, tag="hh")
            nc.vector.tensor_scalar(hh[:, 0:1], hx0, inv[:], None, A.mult)   # hx
            nc.vector.tensor_scalar(hh[:, 1:2], hy0, inv[:], None, A.mult)   # hy
            nc.vector.tensor_scalar(hh[:, 2:3], hh[:, 1:2], -1.0, None, A.mult)  # -hy

            bu = wpool.tile([128, P], dt.float32, tag="bu")
            nc.vector.tensor_scalar(bu[:], uu[:], Lat, None, A.mult)
            bv = wpool.tile([128, P], dt.float32, tag="bv")
            nc.vector.tensor_scalar(bv[:], vv[:], Wat, None, A.mult)
            t1 = wpool.tile([128, P], dt.float32, tag="t1")
            nc.vector.tensor_scalar(t1[:], bu[:], hh[:, 0:1], None, A.mult)
            ox = wpool.tile([128, P], dt.float32, tag="ox")
            nc.vector.scalar_tensor_tensor(ox[:], bv[:], hh[:, 2:3], t1[:], A.mult, A.add)
            nc.vector.tensor_scalar(t1[:], bu[:], hh[:, 1:2], None, A.mult)
            oy = wpool.tile([128, P], dt.float32, tag="oy")
            nc.vector.scalar_tensor_tensor(oy[:], bv[:], hh[:, 0:1], t1[:], A.mult, A.add)

            d2 = wpool.tile([128, P], dt.float32, tag="d2")
            nc.vector.tensor_tensor(d2[:], ox[:], ox[:], A.mult)
            nc.vector.tensor_tensor(t1[:], oy[:], oy[:], A.mult)
            nc.vector.tensor_tensor(d2[:], d2[:], t1[:], A.add)

            ixi = wpool.tile([128, P], dt.int32, tag="ixi")
            iyi = wpool.tile([128, P], dt.int32, tag="iyi")
            for (ov, ctr, res) in ((ox, cx, ixi), (oy, cy, iyi)):
                pw = wpool.tile([128, P], dt.float32, tag="pw")
                nc.vector.tensor_scalar(pw[:], ov[:], ctr, None, A.add)      # px
                nc.vector.tensor_scalar(pw[:], pw[:], invdx[:], None, A.mult)
                nc.vector.tensor_scalar(pw[:], pw[:], 0.0, 2047.0, A.max, A.min)
                ci = wpool.tile([128, P], dt.int32, tag="ci")
                nc.vector.tensor_copy(ci[:], pw[:])                          # RNE
                cf = wpool.tile([128, P], dt.float32, tag="cf")
                nc.vector.tensor_copy(cf[:], ci[:])
                ad = wpool.tile([128, P], dt.float32, tag="ad")
                nc.vector.tensor_tensor(ad[:], cf[:], pw[:], A.is_gt)
                adi = wpool.tile([128, P], dt.int32, tag="adi")
                nc.vector.tensor_copy(adi[:], ad[:])
                nc.vector.tensor_tensor(res[:], ci[:], adi[:], A.subtract)

            x16 = wpool.tile([128, P], dt.int32, tag="x16")
            nc.vector.tensor_scalar(x16[:], ixi[:], 4, None, A.logical_shift_right)
            jst = wpool.tile([128, P], dt.int32, tag="jst")
            nc.vector.tensor_scalar(jst[:], ixi[:], 1, 30, A.logical_shift_left, A.bitwise_and)
            jst16 = wpool.tile([128, P], dt.int16, tag="jst16")
            nc.vector.tensor_copy(jst16[:], jst[:])
            y8p = wpool.tile([128, P], dt.int32, tag="y8p")
            nc.vector.tensor_scalar(y8p[:], iyi[:], 4, None, A.logical_shift_right)
            e32 = wpool.tile([128, P], dt.int32, tag="e32")
            nc.vector.scalar_tensor_tensor(e32[:], x16[:], 128, y8p[:], A.mult, A.add)
            e16 = wpool.tile([128, P], dt.int16, tag="e16")
            nc.vector.tensor_copy(e16[:], e32[:])
            ssel = wpool.tile([128, P], dt.int32, tag="ssel")
            nc.vector.tensor_scalar(ssel[:], iyi[:], 3, 1, A.logical_shift_right, A.bitwise_and)
            sself = wpool.tile([128, P], dt.float32, tag="sself")
            nc.vector.tensor_copy(sself[:], ssel[:])
            rbit = wpool.tile([128, P], dt.int32, tag="rbit")
            nc.vector.tensor_scalar(rbit[:], iyi[:], 7, None, A.bitwise_and)

            g1 = gpool.tile([128, 16 * P * 2], dt.float16, tag="g1")
            nc.gpsimd.ap_gather(g1[:], tab[:], e16[:], channels=128,
                                num_elems=16384, d=2, num_idxs=16 * P)
            g2 = gpool.tile([128, 16 * P * 2], dt.float16, tag="g2")
            nc.gpsimd.ap_gather(g2[:], mt[:], jst16[:], channels=128,
                                num_elems=32, d=2, num_idxs=16 * P)
            nc.vector.tensor_tensor(g1[:], g1[:], g2[:], A.mult)

            # block-diagonal reduce: psum[g, i] = sum_{k in group g} g1[k, i]
            wsel = wpool.tile([128, P * 2], dt.float32, tag="wsel")
            stg = spool.tile([8, 16 * P * 2], dt.float32, tag="stg")
            CH = 400
            for c in range(0, 16 * P * 2, CH):
                pt = ppool.tile([8, CH], dt.float32, tag="pt")
                nc.tensor.matmul(pt[:], ones8[:], g1[:, c:c + CH], start=True, stop=True)
                nc.scalar.copy(stg[:, c:c + CH], pt[:])
            # repartition: stg[g, (s*16+j)*2+b] -> wsel[16g+j, s*2+b]
            src4 = stg[:].rearrange("g (s j b) -> g j s b", s=P, j=16, b=2)
            dst4 = wsel[:].rearrange("(gg j) f -> gg j f", gg=8, j=16)
            for j in range(16):
                nc.sync.dma_start(dst4[:, j, :], src4[:, j, :, :])

            # s-select word pair -> w; integer bit extract
            w0 = wsel[:].rearrange("p (s b) -> p s b", b=2)[:, :, 0:1].rearrange("p s b -> p (s b)")
            w1 = wsel[:].rearrange("p (s b) -> p s b", b=2)[:, :, 1:2].rearrange("p s b -> p (s b)")
            wd = wpool.tile([128, P], dt.float32, tag="wd")
            nc.vector.tensor_tensor(wd[:], w1, w0, A.subtract)
            nc.vector.tensor_tensor(wd[:], wd[:], sself[:], A.mult)
            nc.vector.tensor_tensor(wd[:], wd[:], w0, A.add)
            wi = wpool.tile([128, P], dt.int32, tag="wi")
            nc.vector.tensor_copy(wi[:], wd[:])
            nc.vector.tensor_tensor(wi[:], wi[:], rbit[:], A.logical_shift_right)
            nc.vector.tensor_scalar(wi[:], wi[:], 1, None, A.bitwise_and)

            key = wpool.tile([128, P], dt.float32, tag="key")
            nc.vector.scalar_tensor_tensor(key[:], wi[:], 1e30, d2[:], A.mult, A.add)
            nc.vector.tensor_scalar(key[:], key[:], -1.0, None, A.mult)
            mx8 = wpool.tile([128, 8], dt.float32, tag="mx8")
            nc.vector.max(mx8[:], key[:])
            mi8 = wpool.tile([128, 8], dt.uint32, tag="mi8")
            nc.vector.max_index(mi8[:], mx8[:], key[:])

            mk = wpool.tile([128, 1], dt.float32, tag="mk")
            nc.vector.tensor_scalar(mk[:], mx8[:, 0:1], -1.0, None, A.mult)
            idxf = wpool.tile([128, 1], dt.float32, tag="idxf")
            nc.vector.tensor_copy(idxf[:], mi8[:, 0:1])
            sel = wpool.tile([128, P], dt.float32, tag="sel")
            nc.vector.tensor_scalar(sel[:], iotaf[:], idxf[:], None, A.is_equal)
            oxs = wpool.tile([128, 1], dt.float32, tag="oxs")
            nc.vector.scalar_tensor_tensor(t1[:], ox[:], 1.0, sel[:], A.mult, A.mult,
                                           accum_out=oxs[:])
            oys = wpool.tile([128, 1], dt.float32, tag="oys")
            nc.vector.scalar_tensor_tensor(t1[:], oy[:], 1.0, sel[:], A.mult, A.mult,
                                           accum_out=oys[:])

            fin = wpool.tile([128, 12], dt.float32, tag="fin")
            nc.vector.tensor_tensor(fin[:, 0:1], cx, oxs[:], A.add)       # cxs
            nc.vector.tensor_tensor(fin[:, 1:2], cy, oys[:], A.add)
            nc.vector.tensor_tensor(fin[:, 0:1], cx, fin[:, 0:1], A.subtract)  # dxx
            nc.vector.tensor_tensor(fin[:, 1:2], cy, fin[:, 1:2], A.subtract)
            nc.vector.tensor_tensor(fin[:, 0:1], fin[:, 0:1], fin[:, 0:1], A.mult)
            nc.vector.tensor_tensor(fin[:, 1:2], fin[:, 1:2], fin[:, 1:2], A.mult)
            nc.vector.tensor_tensor(fin[:, 2:3], fin[:, 0:1], fin[:, 1:2], A.add)  # w2
            # dist = sqrt(w2): act-sqrt + one Newton step with exact recip
            nc.scalar.activation(fin[:, 3:4], fin[:, 2:3], mybir.ActivationFunctionType.Sqrt)
            nc.vector.reciprocal(fin[:, 4:5], fin[:, 3:4])
            nc.vector.tensor_tensor(fin[:, 4:5], fin[:, 2:3], fin[:, 4:5], A.mult)
            nc.vector.tensor_tensor(fin[:, 4:5], fin[:, 4:5], fin[:, 3:4], A.add)
            nc.vector.tensor_scalar(fin[:, 3:4], fin[:, 4:5], 0.5, None, A.mult)   # dist
            # pen = sqrt(L^2/4 + W^2/4), same refinement
            nc.vector.tensor_tensor(fin[:, 5:6], Lat, Lat, A.mult)
            nc.vector.tensor_scalar(fin[:, 5:6], fin[:, 5:6], 0.25, None, A.mult)
            nc.vector.tensor_tensor(fin[:, 6:7], Wat, Wat, A.mult)
            nc.vector.tensor_scalar(fin[:, 6:7], fin[:, 6:7], 0.25, None, A.mult)
            nc.vector.tensor_tensor(fin[:, 5:6], fin[:, 5:6], fin[:, 6:7], A.add)
            nc.scalar.activation(fin[:, 6:7], fin[:, 5:6], mybir.ActivationFunctionType.Sqrt)
            nc.vector.reciprocal(fin[:, 7:8], fin[:, 6:7])
            nc.vector.tensor_tensor(fin[:, 7:8], fin[:, 5:6], fin[:, 7:8], A.mult)
            nc.vector.tensor_tensor(fin[:, 7:8], fin[:, 7:8], fin[:, 6:7], A.add)
            nc.vector.tensor_scalar(fin[:, 6:7], fin[:, 7:8], 0.5, None, A.mult)   # pen
            nc.vector.reciprocal(fin[:, 7:8], fin[:, 6:7])
            nc.vector.tensor_tensor(fin[:, 8:9], fin[:, 3:4], fin[:, 7:8], A.mult)
            nc.vector.tensor_scalar(fin[:, 8:9], fin[:, 8:9], -1.0, 1.0, A.mult, A.add)
            nc.vector.tensor_scalar(fin[:, 9:10], mk[:], 1e29, None, A.is_lt)
            nc.vector.tensor_tensor(fin[:, 10:11], fin[:, 8:9], fin[:, 9:10], A.mult)
            nc.sync.dma_start(out_dram[it, :], fin[:, 10:11])

    nc.compile()
    return nc


def kernel(traj, veh_att, raster, mapixes, dx, _trace=False):
    _install_ntff_hook()
    from concourse.bass_utils import run_bass_kernel_spmd

    traj = np.ascontiguousarray(traj, np.float32)
    veh_att = np.ascontiguousarray(veh_att, np.float32)
    raster = np.ascontiguousarray(raster, np.float32)
    mapixes = np.ascontiguousarray(mapixes).astype(np.int64)
    dxf = np.float32(np.asarray(dx).reshape(-1)[0])

    # ---- host layout prep ----
    # Y8 pack: words[m, y8, x] in [0, 256)
    r8 = (raster >= 0.5).astype(np.uint16).reshape(N_MAPS, MAP_H // 8, 8, MAP_W)
    wts = (1 << np.arange(8)).astype(np.uint16)
    words = (r8 * wts[None, None, :, None]).sum(axis=2).astype(np.float16)  # [4,256,2048]

    # per-map ap_gather table [128, 16384, 2]: tab[p, x16*128+y8p, s] =
    #   words[m, (y8p*2+s... careful: e=(x16)*128+(iy>>4); d-pair = words y8=(iy>>3)
    # pair index y8p = iy//16; within pair s=(iy>>3)&1 -> y8 = y8p*2+s
    tabs = []
    for m in range(N_MAPS):
        wm = words[m]  # [256, 2048]
        t = np.zeros((128, 16384, 2), np.float16)
        j = (np.arange(128) % 16)
        x16 = np.arange(128)
        y8p = np.arange(128)
        # t[p, x16*128 + y8p, s] = wm[y8p*2+s, x16*16 + p%16]
        xx = (x16[:, None] * 16)[None, :, :] + j[:, None, None]      # [128p,128x16,1]->x
        for s in range(2):
            # index arrays: [128, 128x16, 128y8p]
            t[:, :, s].reshape(128, 128, 128)[:, :, :] = \
                wm[(y8p * 2 + s)[None, None, :], xx]
        tabs.append(t.reshape(128, 16384 * 2))

    mt = np.zeros((128, 32, 2), np.float16)
    mt[np.arange(128), 2 * (np.arange(128) % 16), :] = 1
    mt = mt.reshape(128, 64)
    ones8 = np.zeros((128, 8), np.float16)
    ones8[np.arange(128), np.arange(128) // 16] = 1

    uu2, vv2 = np.meshgrid(_UU10, _VV20, indexing="ij")
    uu_rep = np.broadcast_to(uu2.reshape(1, P), (128, P)).astype(np.float32).copy()
    vv_rep = np.broadcast_to(vv2.reshape(1, P), (128, P)).astype(np.float32).copy()
    iotaf = np.broadcast_to(np.arange(P, dtype=np.float32)[None, :], (128, P)).copy()
    dxrep = np.full((128, 1), dxf, np.float32)

    # ---- shard agents by map, 2 cores per map ----
    core_agents = [[] for _ in range(N_CORES)]
    for m in range(N_MAPS):
        ags = np.where(mapixes == m)[0]
        half = (len(ags) + 1) // 2
        core_agents[2 * m] = list(ags[:half])
        core_agents[2 * m + 1] = list(ags[half:])

    n_states = [len(a) * T for a in core_agents]
    n_tiles = max(1, int(np.ceil(max(n_states) / 128)))
    S = n_tiles * 128

    in_maps = []
    state_maps = []
    for c in range(N_CORES):
        ags = core_agents[c]
        tr = np.zeros((S, 4), np.float32)
        at = np.zeros((S, 2), np.float32)
        smap = np.full(S, -1, np.int64)
        if ags:
            idx = np.array([(a * T + t) for a in ags for t in range(T)])
            tr[:len(idx)] = traj.reshape(NA * T, 4)[idx]
            at[:len(idx)] = veh_att[np.repeat(ags, T)]
            smap[:len(idx)] = idx
        # pad rows: safe in-bounds values
        pad = smap < 0
        tr[pad] = np.array([100.0, 100.0, 1.0, 0.0], np.float32)
        at[pad] = np.array([4.0, 2.0], np.float32)
        in_maps.append({
            "tab": tabs[c // 2], "mt": mt, "ones8": ones8, "uu": uu_rep,
            "vv": vv_rep, "iotaf": iotaf, "dxrep": dxrep,
            "trajsh": tr, "attsh": at,
        })
        state_maps.append(smap)

    if n_tiles not in _PROGRAM_CACHE:
        _PROGRAM_CACHE[n_tiles] = _build_program(n_tiles)
    nc = _PROGRAM_CACHE[n_tiles]

    try:
        res = run_bass_kernel_spmd(nc, in_maps, list(range(N_CORES)), trace=_trace)
    except Exception:
        if not _trace:
            raise
        res = run_bass_kernel_spmd(nc, in_maps, list(range(N_CORES)), trace=False)
    kernel.last_results = res

    out = np.zeros(NA * T, np.float32)
    for c in range(N_CORES):
        o = res.results[c]["outsh"].reshape(-1)
        valid = state_maps[c] >= 0
        out[state_maps[c][valid]] = o[valid]
    return out

